# revision 7
# baseline (speedup 1.0000x reference)
"""DeepIRT (nn_DeepIRT) Trainium2 Bass kernel.

Full inputs in, full output out. Internally: data-parallel over batch across
8 NeuronCores (16 batch rows per core), everything else on-device per core:

  phase 1: embedding gathers (indirect DMA), PE matmuls for logits/e/a,
           softmax, and gamma = a/e computed division-free as a*(1+exp(-z))
           (e = sigmoid(z) => 1/e = 1 + exp(-z)).
  phase 2: the sequential memory update
               Mv_t = Mv_{t-1} * (1 - w x e) + w x a
           is an elementwise *linear* recurrence per (b,m,d) cell, computed
           with the native DVE prefix-scan (tensor_tensor_scan).  The state
           shift  s~ = Mv - gamma  (gamma = a/e, per (b,d,t) only) makes the
           additive term independent of m:
               s~_t = s~_{t-1} * (1 - w_t e_t) + (gamma_{t-1} - gamma_t)
           and since softmax weights sum to one:
               read_t = sum_m w_t[m] s~_{t-1}[m,d] + gamma_{t-1}[d].
           Chains (one per (d,m) pair, 400 per partition) are packed along
           the free dim with one reset slot per chain per time-chunk to
           carry the state across chunks.
  phase 4: f/ability/que_diff matmuls on PE + final sigmoid.
"""

import math
import os
import sys

import numpy as np

for _p in ("/opt/trn_rl_repo",):
    if _p not in sys.path and os.path.isdir(_p):
        sys.path.insert(0, _p)

import concourse.bass as bass
import concourse.tile as tile
from concourse import mybir
from concourse._compat import with_exitstack
from concourse.bass import IndirectOffsetOnAxis
from concourse.masks import make_identity

AF = mybir.ActivationFunctionType
OP = mybir.AluOpType
FP32 = mybir.dt.float32
I32 = mybir.dt.int32

P = 128  # SBUF partitions


class Cfg:
    def __init__(self, numc=10000, d=64, m=50, l=500, t=25, ncores=8, b=128):
        self.numc, self.d, self.m, self.l, self.t = numc, d, m, l, t
        self.ncores, self.b = ncores, b
        self.bc = b // ncores                      # batch rows per core (16)
        assert P % self.bc == 0
        self.dblk = P // self.bc                   # d-blocks per batch row (8)
        assert d % self.dblk == 0
        self.dsub = d // self.dblk                 # d's per partition (8)
        self.nt = self.bc * l                      # tokens per core
        self.ntile = math.ceil(self.nt / P)
        self.ntp = self.ntile * P
        self.cl = t + 1                            # chain slots per chunk
        assert l % t == 0
        self.nch = l // t
        self.gc = l + 4                            # padded per-b gamma cols
        self.fbig = self.dsub * m * self.cl        # scan buffer elems/partition


CFG = Cfg()


def _split_tokens(g0, n, l):
    """Split token range [g0, g0+n) at batch-row boundaries.

    Yields (b, tlo, thi, j0): tokens g0+j0 .. map to (b, tlo..thi)."""
    j = 0
    while j < n:
        g = g0 + j
        b, t = divmod(g, l)
        cnt = min(n - j, l - t)
        yield b, t, t + cnt, j
        j += cnt


@with_exitstack
def build_deepirt(ctx, tc, io, cfg):
    nc = tc.nc
    D, M, L, T, BC = cfg.d, cfg.m, cfg.l, cfg.t, cfg.bc
    DBLK, DSUB, CL, NCH, GC = cfg.dblk, cfg.dsub, cfg.cl, cfg.nch, cfg.gc
    NT, NTILE, NTP = cfg.nt, cfg.ntile, cfg.ntp

    # internal DRAM scratch
    w_dram = nc.dram_tensor("w_scr", [NTP, M], FP32).ap()
    e_dram = nc.dram_tensor("e_scr", [D, NTP], FP32).ap()
    g_dram = nc.dram_tensor("g_scr", [D, BC * GC], FP32).ap()
    kT_dram = nc.dram_tensor("kT_scr", [D, NTP], FP32).ap()
    rd_dram = nc.dram_tensor("rd_scr", [D, NTP], FP32).ap()

    g_dram_v = g_dram.rearrange("d (b c) -> d b c", b=BC)

    cpool = ctx.enter_context(tc.tile_pool(name="const", bufs=1))
    ident = cpool.tile([P, P], FP32, tag="ident")
    make_identity(nc, ident[:])
    ones = cpool.tile([1, P], FP32, tag="ones")
    nc.vector.memset(ones[:], 1.0)
    zeros = cpool.tile([D, P], FP32, tag="zeros")
    nc.vector.memset(zeros[:], 0.0)

    # ---------------- phase 1 ----------------
    with (
        tc.tile_pool(name="p1w", bufs=1) as wp,
        tc.tile_pool(name="p1", bufs=3) as pool,
        tc.tile_pool(name="p1ps", bufs=1, space="PSUM") as pp,
    ):
        mkt_sb = wp.tile([D, M], FP32, tag="mkt")
        nc.sync.dma_start(mkt_sb[:], io["MkT"])
        ew_sb = wp.tile([D, D], FP32, tag="ew")
        nc.sync.dma_start(ew_sb[:], io["eW"])
        aw_sb = wp.tile([D, D], FP32, tag="aw")
        nc.sync.dma_start(aw_sb[:], io["aW"])
        eb_sb = wp.tile([1, D], FP32, tag="eb")
        nc.sync.dma_start(eb_sb[:], io["eb"])
        ab_sb = wp.tile([1, D], FP32, tag="ab")
        nc.sync.dma_start(ab_sb[:], io["ab"])

        # zero pads: gamma tail cols per b, kT/reads pad cols
        nc.sync.dma_start(g_dram_v[:, :, L:GC], zeros[:, : BC * (GC - L)].rearrange("d (b c) -> d b c", b=BC))
        if NTP > NT:
            nc.sync.dma_start(kT_dram[:, NT:NTP], zeros[:, : NTP - NT])
            nc.sync.dma_start(rd_dram[:, NT:NTP], zeros[:, : NTP - NT])

        for i in range(NTILE):
            g0 = i * P
            n = min(P, NT - g0)
            idx_t = pool.tile([P, 2], I32, tag="idx")
            nc.sync.dma_start(idx_t[:], io["kvidx"][g0 : g0 + P, :])
            k_tok = pool.tile([P, D], FP32, tag="ktok")
            nc.gpsimd.indirect_dma_start(
                out=k_tok[:], out_offset=None, in_=io["k_emb"],
                in_offset=IndirectOffsetOnAxis(ap=idx_t[:, 0:1], axis=0),
            )
            v_tok = pool.tile([P, D], FP32, tag="vtok")
            nc.gpsimd.indirect_dma_start(
                out=v_tok[:], out_offset=None, in_=io["v_emb"],
                in_offset=IndirectOffsetOnAxis(ap=idx_t[:, 1:2], axis=0),
            )
            kt_ps = pp.tile([D, P], FP32, tag="ktps")
            # transpose as a plain matmul k_tok.T @ I (the dedicated transpose
            # lowers to an LW-struct that only carries one sync wait)
            nc.tensor.matmul(kt_ps[:], lhsT=k_tok[:], rhs=ident[:], start=True, stop=True)
            kt = pool.tile([D, P], FP32, tag="kt")
            nc.scalar.copy(kt[:], kt_ps[:])
            vt_ps = pp.tile([D, P], FP32, tag="vtps")
            nc.tensor.matmul(vt_ps[:], lhsT=v_tok[:], rhs=ident[:], start=True, stop=True)
            vt = pool.tile([D, P], FP32, tag="vt")
            nc.scalar.copy(vt[:], vt_ps[:])
            nc.sync.dma_start(kT_dram[:, g0 : g0 + P], kt[:])

            # softmax weights (token-major)
            lg_ps = pp.tile([P, M], FP32, tag="lg")
            nc.tensor.matmul(lg_ps[:], lhsT=kt[:], rhs=mkt_sb[:], start=True, stop=True)
            wexp = pool.tile([P, M], FP32, tag="wexp")
            wsum = pool.tile([P, 1], FP32, tag="wsum")
            nc.scalar.activation(wexp[:], lg_ps[:], AF.Exp, accum_out=wsum[:])
            wrec = pool.tile([P, 1], FP32, tag="wrec")
            nc.vector.reciprocal(wrec[:], wsum[:])
            wn = pool.tile([P, M], FP32, tag="wn")
            nc.scalar.activation(wn[:], wexp[:], AF.Copy, scale=wrec[:])
            nc.sync.dma_start(w_dram[g0 : g0 + n, :], wn[:n, :])

            # e / a / gamma (feature-major)
            e_ps = pp.tile([D, P], FP32, tag="eps")
            nc.tensor.matmul(e_ps[:], lhsT=ew_sb[:], rhs=vt[:], start=True, stop=False)
            nc.tensor.matmul(e_ps[:], lhsT=eb_sb[:], rhs=ones[:], start=False, stop=True)
            e_sb = pool.tile([D, P], FP32, tag="esb")
            nc.scalar.activation(e_sb[:], e_ps[:], AF.Sigmoid)
            xz = pool.tile([D, P], FP32, tag="xz")
            nc.scalar.activation(xz[:], e_ps[:], AF.Exp, scale=-1.0)
            a_ps = pp.tile([D, P], FP32, tag="aps")
            nc.tensor.matmul(a_ps[:], lhsT=aw_sb[:], rhs=vt[:], start=True, stop=False)
            nc.tensor.matmul(a_ps[:], lhsT=ab_sb[:], rhs=ones[:], start=False, stop=True)
            a_sb = pool.tile([D, P], FP32, tag="asb")
            nc.scalar.activation(a_sb[:], a_ps[:], AF.Tanh)
            g_sb = pool.tile([D, P], FP32, tag="gsb")
            nc.vector.scalar_tensor_tensor(
                out=g_sb[:], in0=xz[:], scalar=1.0, in1=a_sb[:],
                op0=OP.add, op1=OP.mult,
            )
            nc.sync.dma_start(e_dram[:, g0 : g0 + n], e_sb[:, :n])
            for b, tlo, thi, j0 in _split_tokens(g0, n, L):
                nc.sync.dma_start(g_dram_v[:, b, tlo:thi], g_sb[:, j0 : j0 + (thi - tlo)])

    # ---------------- phase 2: the scan ----------------
    with tc.tile_pool(name="scan", bufs=1) as sp:
        A = sp.tile([P, cfg.fbig], FP32, tag="A", name="A")
        DL = [sp.tile([P, cfg.fbig], FP32, tag=f"DL{i}", name=f"DL{i}") for i in range(2)]
        S = sp.tile([P, cfg.fbig], FP32, tag="S", name="S")
        wt = [sp.tile([P, T * M], FP32, tag=f"wt{i}", name=f"wt{i}") for i in range(2)]
        et = [sp.tile([P, DSUB * T], FP32, tag=f"et{i}", name=f"et{i}") for i in range(2)]
        gt = [sp.tile([P, DSUB * CL], FP32, tag=f"gt{i}", name=f"gt{i}") for i in range(2)]
        ds = [sp.tile([P, DSUB * T], FP32, tag=f"ds{i}", name=f"ds{i}") for i in range(2)]
        rr = [sp.tile([P, DSUB * T], FP32, tag=f"rr{i}", name=f"rr{i}") for i in range(2)]
        mv = sp.tile([P, DSUB * M], FP32, tag="mv")
        g00 = sp.tile([P, DSUB], FP32, tag="g00")
        carry0 = sp.tile([P, DSUB * M], FP32, tag="c0")

        # zero the per-chain reset slots of A (slot 0 of each chain)
        nc.vector.memset(A[:].rearrange("p (c l) -> p c l", l=CL)[:, :, 0:1], 0.0)

        # Mv0 into (b, dblk) partitions, free layout (d_loc, m)
        mv0_v = io["Mv0"].rearrange("m (g s) -> g s m", s=DSUB)  # [DBLK, DSUB, M]
        for b in range(BC):
            nc.sync.dma_start(
                mv[b * DBLK : (b + 1) * DBLK, :].rearrange("p (s m) -> p s m", m=M),
                mv0_v,
            )
            # gamma_0 = gamma col 0 of this b
            nc.sync.dma_start(
                g00[b * DBLK : (b + 1) * DBLK, :],
                g_dram_v[:, b, 0:1].rearrange("(g s) x -> g (s x)", s=DSUB),
            )
        nc.vector.scalar_tensor_tensor(
            out=carry0[:].rearrange("p (s m) -> p s m", m=M),
            in0=mv[:].rearrange("p (s m) -> p s m", m=M),
            scalar=1.0,
            in1=g00[:].unsqueeze(2).broadcast_to([P, DSUB, M]),
            op0=OP.mult, op1=OP.subtract,
        )

        for ch in range(NCH):
            cur = ch % 2
            t0 = ch * T
            # ---- chunk loads
            # w: [P, (j, m)] (replicated over dblk partitions via step-0)
            w_src = bass.AP(
                w_dram.tensor, w_dram.offset + t0 * M,
                [[L * M, BC], [0, DBLK], [1, T * M]],
            )
            nc.sync.dma_start(wt[cur][:].rearrange("p (f) -> p f"), w_src)
            # e: [P, (d_loc, j)]
            e_src = bass.AP(
                e_dram.tensor, e_dram.offset + t0,
                [[L, BC], [DSUB * NTP, DBLK], [NTP, DSUB], [1, T]],
            )
            nc.sync.dma_start(et[cur][:].rearrange("p (s j) -> p s j", j=T), e_src)
            # gamma: [P, (d_loc, j)] cols t0 .. t0+T (CL of them)
            g_src = bass.AP(
                g_dram.tensor, g_dram.offset + t0,
                [[GC, BC], [DSUB * BC * GC, DBLK], [BC * GC, DSUB], [1, CL]],
            )
            nc.sync.dma_start(gt[cur][:].rearrange("p (s j) -> p s j", j=CL), g_src)

            A_4 = A[:].rearrange("p (s m l) -> p s m l", m=M, l=CL)
            DL_4 = DL[cur][:].rearrange("p (s m l) -> p s m l", m=M, l=CL)
            S_4 = S[:].rearrange("p (s m l) -> p s m l", m=M, l=CL)
            w_mj = wt[cur][:].rearrange("p (j m) -> p m j", m=M)
            e_sj = et[cur][:].rearrange("p (s j) -> p s j", j=T)
            g_v = gt[cur][:].rearrange("p (s j) -> p s j", j=CL)

            # carry slot (slot 0 of every chain) of data1: for chunk 0 it is
            # written before the loop; for later chunks at the end of the
            # previous chunk (from S's last slots).
            if ch == 0:
                dl_slot0 = DL[0][:].rearrange("p (c l) -> p c l", l=CL)[:, :, 0:1]
                nc.scalar.copy(dl_slot0, carry0[:].unsqueeze(2))

            # A = 1 - w*e   (stt computes w*e per d_loc; ACT flips to 1 - x
            # in place over the whole buffer).  walrus limits stt/ACT APs to
            # 2 free dims, hence the per-d_loc split.
            for s in range(DSUB):
                nc.vector.scalar_tensor_tensor(
                    out=A_4[:, s, :, 1:], in0=w_mj, scalar=1.0,
                    in1=e_sj[:, s : s + 1, :].broadcast_to([P, M, T]),
                    op0=OP.mult, op1=OP.mult,
                )
            A_slots = A[:].rearrange("p (c l) -> p c l", l=CL)[:, :, 1:]
            nc.scalar.activation(A_slots, A_slots, AF.Copy, scale=-1.0, bias=1.0)

            # delta = gamma_{t-1} - gamma_t, replicated over m into data1
            ds_v = ds[cur][:].rearrange("p (s j) -> p s j", j=T)
            nc.vector.tensor_tensor(
                out=ds_v, in0=g_v[:, :, 0:T], in1=g_v[:, :, 1:CL], op=OP.subtract
            )
            for s in range(DSUB):
                nc.scalar.copy(
                    DL_4[:, s, :, 1:],
                    ds_v[:, s : s + 1, :].broadcast_to([P, M, T]),
                )

            # the scan
            nc.vector.tensor_tensor_scan(
                out=S[:], data0=A[:], data1=DL[cur][:],
                initial=0.0, op0=OP.mult, op1=OP.add,
            )

            # carry for the next chunk: last slot of each chain -> slot 0 of
            # the other DL buffer
            if ch < NCH - 1:
                dl_next0 = DL[1 - cur][:].rearrange("p (c l) -> p c l", l=CL)[:, :, 0:1]
                s_last = S[:].rearrange("p (c l) -> p c l", l=CL)[:, :, T : T + 1]
                nc.scalar.copy(dl_next0, s_last)

            # read_t = sum_m w_t * s~_{t-1} + gamma_{t-1}
            for s in range(DSUB):
                ws_s = DL[cur][:, s * M * T : (s + 1) * M * T].rearrange(
                    "p (m j) -> p m j", j=T
                )
                nc.vector.tensor_tensor(
                    out=ws_s, in0=S_4[:, s, :, 0:T], in1=w_mj, op=OP.mult
                )
            ws_r = DL[cur][:, : DSUB * M * T].rearrange("p (s m j) -> p s j m", m=M, j=T)
            rr_v = rr[cur][:].rearrange("p (s j) -> p s j", j=T)
            nc.vector.tensor_reduce(out=rr_v, in_=ws_r, axis=mybir.AxisListType.X, op=OP.add)
            nc.vector.tensor_tensor(out=rr_v, in0=rr_v, in1=g_v[:, :, 0:T], op=OP.add)

            rd_dst = bass.AP(
                rd_dram.tensor, rd_dram.offset + t0,
                [[L, BC], [DSUB * NTP, DBLK], [NTP, DSUB], [1, T]],
            )
            nc.sync.dma_start(rd_dst, rr_v)

    # ---------------- phase 4 ----------------
    with (
        tc.tile_pool(name="p4w", bufs=1) as wp4,
        tc.tile_pool(name="p4", bufs=3) as pool4,
        tc.tile_pool(name="p4ps", bufs=2, space="PSUM") as pp4,
    ):
        fwr_sb = wp4.tile([D, D], FP32, tag="fwr")
        nc.sync.dma_start(fwr_sb[:], io["fWr"])
        fwk_sb = wp4.tile([D, D], FP32, tag="fwk")
        nc.sync.dma_start(fwk_sb[:], io["fWk"])
        fb_sb = wp4.tile([1, D], FP32, tag="fb")
        nc.sync.dma_start(fb_sb[:], io["fb"])
        abw_sb = wp4.tile([D, 1], FP32, tag="abw")
        nc.sync.dma_start(abw_sb[:], io["abW"])
        dw_sb = wp4.tile([D, 1], FP32, tag="dw")
        nc.sync.dma_start(dw_sb[:], io["dW"])
        abb_sb = wp4.tile([P, 1], FP32, tag="abb")
        nc.sync.dma_start(abb_sb[:], io["abb"])
        dbb_sb = wp4.tile([P, 1], FP32, tag="dbb")
        nc.sync.dma_start(dbb_sb[:], io["dbb"])

        for i in range(NTILE):
            g0 = i * P
            kt4 = pool4.tile([D, P], FP32, tag="kt4")
            nc.sync.dma_start(kt4[:], kT_dram[:, g0 : g0 + P])
            rt4 = pool4.tile([D, P], FP32, tag="rt4")
            nc.sync.dma_start(rt4[:], rd_dram[:, g0 : g0 + P])
            f_ps = pp4.tile([D, P], FP32, tag="fps")
            nc.tensor.matmul(f_ps[:], lhsT=fwr_sb[:], rhs=rt4[:], start=True, stop=False)
            nc.tensor.matmul(f_ps[:], lhsT=fwk_sb[:], rhs=kt4[:], start=False, stop=False)
            nc.tensor.matmul(f_ps[:], lhsT=fb_sb[:], rhs=ones[:], start=False, stop=True)
            f_sb = pool4.tile([D, P], FP32, tag="fsb")
            nc.scalar.activation(f_sb[:], f_ps[:], AF.Tanh)
            ab_ps = pp4.tile([P, 1], FP32, tag="abps")
            nc.tensor.matmul(ab_ps[:], lhsT=f_sb[:], rhs=abw_sb[:], start=True, stop=True)
            qd_ps = pp4.tile([P, 1], FP32, tag="qdps")
            nc.tensor.matmul(qd_ps[:], lhsT=kt4[:], rhs=dw_sb[:], start=True, stop=True)
            abil = pool4.tile([P, 1], FP32, tag="abil")
            nc.scalar.activation(abil[:], ab_ps[:], AF.Tanh, bias=abb_sb[:])
            qd = pool4.tile([P, 1], FP32, tag="qd")
            nc.scalar.activation(qd[:], qd_ps[:], AF.Tanh, bias=dbb_sb[:])
            z = pool4.tile([P, 1], FP32, tag="z")
            nc.vector.scalar_tensor_tensor(
                out=z[:], in0=abil[:], scalar=3.0, in1=qd[:],
                op0=OP.mult, op1=OP.subtract,
            )
            pt = pool4.tile([P, 1], FP32, tag="pt")
            nc.scalar.activation(pt[:], z[:], AF.Sigmoid)
            nc.sync.dma_start(io["p_out"][g0 : g0 + P, :], pt[:])


def _split_multi_waits(nc):
    """This walrus build allows only ONE sync-wait per instruction; move
    extras onto standalone InstEventSemaphore ops just before the
    instruction on the same engine (raw-bass style standalone waits)."""
    n = 0
    for fn in nc.m.functions:
        for blk in fn.blocks:
            new_list = []
            for inst in blk.instructions:
                si = inst.sync_info
                if si is not None and si.on_wait and len(si.on_wait) > 1:
                    for w in si.on_wait[:-1]:
                        n += 1
                        ev = mybir.InstEventSemaphore(
                            name=f"xwait_{n}_{inst.name}", ins=[], outs=[],
                            sync_info=mybir.SyncInfo(on_wait=[w], on_update=[]),
                        )
                        ev.engine = inst.engine
                        nc.register_instruction(ev, overwrite=True)
                        new_list.append(ev)
                    inst.sync_info = mybir.SyncInfo(
                        on_wait=[si.on_wait[-1]], on_update=si.on_update
                    )
                new_list.append(inst)
            blk.instructions[:] = new_list
    return n


def declare_io(nc, cfg):
    io = {}

    def inp(name, shape, dt=FP32):
        io[name] = nc.dram_tensor(name, shape, dt, kind="ExternalInput").ap()

    inp("kvidx", [cfg.ntp, 2], I32)
    inp("k_emb", [cfg.numc, cfg.d])
    inp("v_emb", [2 * cfg.numc, cfg.d])
    inp("MkT", [cfg.d, cfg.m])
    inp("eW", [cfg.d, cfg.d])
    inp("aW", [cfg.d, cfg.d])
    inp("eb", [1, cfg.d])
    inp("ab", [1, cfg.d])
    inp("fWr", [cfg.d, cfg.d])
    inp("fWk", [cfg.d, cfg.d])
    inp("fb", [1, cfg.d])
    inp("abW", [cfg.d, 1])
    inp("dW", [cfg.d, 1])
    inp("abb", [P, 1])
    inp("dbb", [P, 1])
    inp("Mv0", [cfg.m, cfg.d])
    io["p_out"] = nc.dram_tensor("p_out", [cfg.ntp, 1], FP32, kind="ExternalOutput").ap()
    return io


def build_nc(cfg=CFG):
    nc = bass.Bass("TRN2", num_devices=cfg.ncores)
    with tile.TileContext(nc) as tc:
        io = declare_io(nc, cfg)
        build_deepirt(tc, io, cfg)
    _split_multi_waits(nc)
    return nc


def host_prep(cfg, q, r, k_emb, v_emb, Mk, Mv0, e_W, e_b, a_W, a_b, f_W, f_b,
              ab_W, ab_b, d_W, d_b):
    """Returns per-core input maps."""
    q = np.asarray(q)
    r = np.asarray(r)
    shared = {
        "k_emb": np.ascontiguousarray(k_emb, np.float32),
        "v_emb": np.ascontiguousarray(v_emb, np.float32),
        "MkT": np.ascontiguousarray(np.asarray(Mk, np.float32).T),
        "eW": np.ascontiguousarray(e_W, np.float32),
        "aW": np.ascontiguousarray(a_W, np.float32),
        "eb": np.asarray(e_b, np.float32).reshape(1, cfg.d),
        "ab": np.asarray(a_b, np.float32).reshape(1, cfg.d),
        "fWr": np.ascontiguousarray(np.asarray(f_W, np.float32)[: cfg.d]),
        "fWk": np.ascontiguousarray(np.asarray(f_W, np.float32)[cfg.d :]),
        "fb": np.asarray(f_b, np.float32).reshape(1, cfg.d),
        "abW": np.asarray(ab_W, np.float32).reshape(cfg.d, 1),
        "dW": np.asarray(d_W, np.float32).reshape(cfg.d, 1),
        "abb": np.full((P, 1), np.float32(np.asarray(ab_b).reshape(-1)[0])),
        "dbb": np.full((P, 1), np.float32(np.asarray(d_b).reshape(-1)[0])),
        "Mv0": np.ascontiguousarray(Mv0, np.float32),
    }
    maps = []
    for c in range(cfg.ncores):
        bsl = slice(c * cfg.bc, (c + 1) * cfg.bc)
        kidx = q[bsl].reshape(-1).astype(np.int32)
        vidx = (q[bsl].astype(np.int64) + cfg.numc * r[bsl].astype(np.int64)).reshape(-1).astype(np.int32)
        kv = np.zeros((cfg.ntp, 2), np.int32)
        kv[: cfg.nt, 0] = kidx
        kv[: cfg.nt, 1] = vidx
        maps.append({"kvidx": kv, **shared})
    return maps


_NC_CACHE = {}


def kernel(**inputs):
    cfg = CFG
    if "nc" not in _NC_CACHE:
        _NC_CACHE["nc"] = build_nc(cfg)
    nc = _NC_CACHE["nc"]
    from concourse.bass_utils import run_bass_kernel_spmd

    maps = host_prep(cfg, **inputs)
    res = run_bass_kernel_spmd(nc, maps, core_ids=list(range(cfg.ncores)))
    outs = []
    for c in range(cfg.ncores):
        p = res.results[c]["p_out"].reshape(-1)[: cfg.nt].reshape(cfg.bc, cfg.l)
        outs.append(p)
    return np.concatenate(outs, axis=0).astype(np.float32)


# revision 9
# speedup vs baseline: 47.3061x; 47.3061x over previous
"""DeepIRT (nn_DeepIRT) Trainium2 Bass kernel.

Full inputs in, full output out. Internally: data-parallel over batch across
8 NeuronCores (16 batch rows per core), everything else on-device per core:

  phase 1: embedding gathers (indirect DMA), PE matmuls for logits/e/a,
           softmax, and gamma = a/e computed division-free as a*(1+exp(-z))
           (e = sigmoid(z) => 1/e = 1 + exp(-z)).
  phase 2: the sequential memory update
               Mv_t = Mv_{t-1} * (1 - w x e) + w x a
           is an elementwise *linear* recurrence per (b,m,d) cell, computed
           with the native DVE prefix-scan (tensor_tensor_scan).  The state
           shift  s~ = Mv - gamma  (gamma = a/e, per (b,d,t) only) makes the
           additive term independent of m:
               s~_t = s~_{t-1} * (1 - w_t e_t) + (gamma_{t-1} - gamma_t)
           and since softmax weights sum to one:
               read_t = sum_m w_t[m] s~_{t-1}[m,d] + gamma_{t-1}[d].
           Chains (one per (d,m) pair, 400 per partition) are packed along
           the free dim with one reset slot per chain per time-chunk to
           carry the state across chunks.
  phase 4: f/ability/que_diff matmuls on PE + final sigmoid.
"""

import math
import os
import sys

import numpy as np

for _p in ("/opt/trn_rl_repo",):
    if _p not in sys.path and os.path.isdir(_p):
        sys.path.insert(0, _p)

import concourse.bass as bass
import concourse.tile as tile
from concourse import mybir
from concourse._compat import with_exitstack
from concourse.bass import IndirectOffsetOnAxis
from concourse.masks import make_identity

AF = mybir.ActivationFunctionType
OP = mybir.AluOpType
FP32 = mybir.dt.float32
I32 = mybir.dt.int32

P = 128  # SBUF partitions


class Cfg:
    def __init__(self, numc=10000, d=64, m=50, l=500, t=25, ncores=8, b=128):
        self.numc, self.d, self.m, self.l, self.t = numc, d, m, l, t
        self.ncores, self.b = ncores, b
        self.bc = b // ncores                      # batch rows per core (16)
        assert P % self.bc == 0
        self.dblk = P // self.bc                   # d-blocks per batch row (8)
        assert d % self.dblk == 0
        self.dsub = d // self.dblk                 # d's per partition (8)
        self.nt = self.bc * l                      # tokens per core
        self.ntile = math.ceil(self.nt / P)
        self.ntp = self.ntile * P
        self.cl = t + 1                            # chain slots per chunk
        assert l % t == 0
        self.nch = l // t
        self.gc = l + 4                            # padded per-b gamma cols
        self.fbig = self.dsub * m * self.cl        # scan buffer elems/partition


CFG = Cfg()


def _split_tokens(g0, n, l):
    """Split token range [g0, g0+n) at batch-row boundaries.

    Yields (b, tlo, thi, j0): tokens g0+j0 .. map to (b, tlo..thi)."""
    j = 0
    while j < n:
        g = g0 + j
        b, t = divmod(g, l)
        cnt = min(n - j, l - t)
        yield b, t, t + cnt, j
        j += cnt


@with_exitstack
def build_deepirt(ctx, tc, io, cfg):
    nc = tc.nc
    D, M, L, T, BC = cfg.d, cfg.m, cfg.l, cfg.t, cfg.bc
    DBLK, DSUB, CL, NCH, GC = cfg.dblk, cfg.dsub, cfg.cl, cfg.nch, cfg.gc
    NT, NTILE, NTP = cfg.nt, cfg.ntile, cfg.ntp

    # internal DRAM scratch
    w_dram = nc.dram_tensor("w_scr", [NTP, M], FP32).ap()
    e_dram = nc.dram_tensor("e_scr", [D, NTP], FP32).ap()
    g_dram = nc.dram_tensor("g_scr", [D, BC * GC], FP32).ap()
    kT_dram = nc.dram_tensor("kT_scr", [D, NTP], FP32).ap()
    rd_dram = nc.dram_tensor("rd_scr", [D, NTP], FP32).ap()

    g_dram_v = g_dram.rearrange("d (b c) -> d b c", b=BC)

    cpool = ctx.enter_context(tc.tile_pool(name="const", bufs=1))
    ident = cpool.tile([P, P], FP32, tag="ident")
    make_identity(nc, ident[:])
    ones = cpool.tile([1, P], FP32, tag="ones")
    nc.vector.memset(ones[:], 1.0)
    zeros = cpool.tile([D, P], FP32, tag="zeros")
    nc.vector.memset(zeros[:], 0.0)

    # ---------------- phase 1 ----------------
    with (
        tc.tile_pool(name="p1w", bufs=1) as wp,
        tc.tile_pool(name="p1", bufs=3) as pool,
        tc.tile_pool(name="p1ps", bufs=1, space="PSUM") as pp,
    ):
        mkt_sb = wp.tile([D, M], FP32, tag="mkt")
        nc.sync.dma_start(mkt_sb[:], io["MkT"])
        ew_sb = wp.tile([D, D], FP32, tag="ew")
        nc.sync.dma_start(ew_sb[:], io["eW"])
        aw_sb = wp.tile([D, D], FP32, tag="aw")
        nc.sync.dma_start(aw_sb[:], io["aW"])
        eb_sb = wp.tile([1, D], FP32, tag="eb")
        nc.sync.dma_start(eb_sb[:], io["eb"])
        ab_sb = wp.tile([1, D], FP32, tag="ab")
        nc.sync.dma_start(ab_sb[:], io["ab"])

        # zero pads: gamma tail cols per b, kT/reads pad cols
        nc.sync.dma_start(g_dram_v[:, :, L:GC], zeros[:, : BC * (GC - L)].rearrange("d (b c) -> d b c", b=BC))
        if NTP > NT:
            nc.sync.dma_start(kT_dram[:, NT:NTP], zeros[:, : NTP - NT])
            nc.sync.dma_start(rd_dram[:, NT:NTP], zeros[:, : NTP - NT])

        for i in range(NTILE):
            g0 = i * P
            n = min(P, NT - g0)
            idx_t = pool.tile([P, 2], I32, tag="idx")
            nc.sync.dma_start(idx_t[:], io["kvidx"][g0 : g0 + P, :])
            k_tok = pool.tile([P, D], FP32, tag="ktok")
            nc.gpsimd.indirect_dma_start(
                out=k_tok[:], out_offset=None, in_=io["k_emb"],
                in_offset=IndirectOffsetOnAxis(ap=idx_t[:, 0:1], axis=0),
            )
            v_tok = pool.tile([P, D], FP32, tag="vtok")
            nc.gpsimd.indirect_dma_start(
                out=v_tok[:], out_offset=None, in_=io["v_emb"],
                in_offset=IndirectOffsetOnAxis(ap=idx_t[:, 1:2], axis=0),
            )
            kt_ps = pp.tile([D, P], FP32, tag="ktps")
            # transpose as a plain matmul k_tok.T @ I (the dedicated transpose
            # lowers to an LW-struct that only carries one sync wait)
            nc.tensor.matmul(kt_ps[:], lhsT=k_tok[:], rhs=ident[:], start=True, stop=True)
            kt = pool.tile([D, P], FP32, tag="kt")
            nc.scalar.copy(kt[:], kt_ps[:])
            vt_ps = pp.tile([D, P], FP32, tag="vtps")
            nc.tensor.matmul(vt_ps[:], lhsT=v_tok[:], rhs=ident[:], start=True, stop=True)
            vt = pool.tile([D, P], FP32, tag="vt")
            nc.scalar.copy(vt[:], vt_ps[:])
            nc.sync.dma_start(kT_dram[:, g0 : g0 + P], kt[:])

            # softmax weights (token-major)
            lg_ps = pp.tile([P, M], FP32, tag="lg")
            nc.tensor.matmul(lg_ps[:], lhsT=kt[:], rhs=mkt_sb[:], start=True, stop=True)
            wexp = pool.tile([P, M], FP32, tag="wexp")
            wsum = pool.tile([P, 1], FP32, tag="wsum")
            nc.scalar.activation(wexp[:], lg_ps[:], AF.Exp, accum_out=wsum[:])
            wrec = pool.tile([P, 1], FP32, tag="wrec")
            nc.vector.reciprocal(wrec[:], wsum[:])
            wn = pool.tile([P, M], FP32, tag="wn")
            nc.scalar.activation(wn[:], wexp[:], AF.Copy, scale=wrec[:])
            nc.sync.dma_start(w_dram[g0 : g0 + n, :], wn[:n, :])

            # e / a / gamma (feature-major)
            e_ps = pp.tile([D, P], FP32, tag="eps")
            nc.tensor.matmul(e_ps[:], lhsT=ew_sb[:], rhs=vt[:], start=True, stop=False)
            nc.tensor.matmul(e_ps[:], lhsT=eb_sb[:], rhs=ones[:], start=False, stop=True)
            # sigmoid via tanh so every ACT func stays in the exp_and_others
            # table (sigmoid and exp never share a table -> reload stalls)
            e_th = pool.tile([D, P], FP32, tag="eth")
            nc.scalar.activation(e_th[:], e_ps[:], AF.Tanh, scale=0.5)
            e_sb = pool.tile([D, P], FP32, tag="esb")
            nc.vector.tensor_scalar(
                out=e_sb[:], in0=e_th[:], scalar1=0.5, scalar2=0.5,
                op0=OP.mult, op1=OP.add,
            )
            xz = pool.tile([D, P], FP32, tag="xz")
            nc.scalar.activation(xz[:], e_ps[:], AF.Exp, scale=-1.0)
            a_ps = pp.tile([D, P], FP32, tag="aps")
            nc.tensor.matmul(a_ps[:], lhsT=aw_sb[:], rhs=vt[:], start=True, stop=False)
            nc.tensor.matmul(a_ps[:], lhsT=ab_sb[:], rhs=ones[:], start=False, stop=True)
            a_sb = pool.tile([D, P], FP32, tag="asb")
            nc.scalar.activation(a_sb[:], a_ps[:], AF.Tanh)
            g_sb = pool.tile([D, P], FP32, tag="gsb")
            nc.vector.scalar_tensor_tensor(
                out=g_sb[:], in0=xz[:], scalar=1.0, in1=a_sb[:],
                op0=OP.add, op1=OP.mult,
            )
            nc.sync.dma_start(e_dram[:, g0 : g0 + n], e_sb[:, :n])
            for b, tlo, thi, j0 in _split_tokens(g0, n, L):
                nc.sync.dma_start(g_dram_v[:, b, tlo:thi], g_sb[:, j0 : j0 + (thi - tlo)])

    # ---------------- phase 2: the scan ----------------
    with tc.tile_pool(name="scan", bufs=1) as sp:
        A = sp.tile([P, cfg.fbig], FP32, tag="A", name="A")
        DL = [sp.tile([P, cfg.fbig], FP32, tag=f"DL{i}", name=f"DL{i}") for i in range(2)]
        S = sp.tile([P, cfg.fbig], FP32, tag="S", name="S")
        wt = [sp.tile([P, T * M], FP32, tag=f"wt{i}", name=f"wt{i}") for i in range(2)]
        et = [sp.tile([P, DSUB * T], FP32, tag=f"et{i}", name=f"et{i}") for i in range(2)]
        gt = [sp.tile([P, DSUB * CL], FP32, tag=f"gt{i}", name=f"gt{i}") for i in range(2)]
        ds = [sp.tile([P, DSUB * T], FP32, tag=f"ds{i}", name=f"ds{i}") for i in range(2)]
        rr = [sp.tile([P, DSUB * T], FP32, tag=f"rr{i}", name=f"rr{i}") for i in range(2)]
        mv = sp.tile([P, DSUB * M], FP32, tag="mv")
        g00 = sp.tile([P, DSUB], FP32, tag="g00")
        carry0 = sp.tile([P, DSUB * M], FP32, tag="c0")

        # zero the per-chain reset slots of A (slot 0 of each chain)
        nc.vector.memset(A[:].rearrange("p (c l) -> p c l", l=CL)[:, :, 0:1], 0.0)

        # Mv0 into (b, dblk) partitions, free layout (d_loc, m); one DMA
        # with a step-0 b-dim broadcasts the table to all batch rows.
        mv0_v = io["Mv0"].rearrange("m (g s) -> g s m", s=DSUB)  # [DBLK, DSUB, M]
        for b in range(BC):
            nc.sync.dma_start(
                mv[b * DBLK : (b + 1) * DBLK, :].rearrange("p (s m) -> p s m", m=M),
                mv0_v,
            )
        g00_src = bass.AP(
            g_dram.tensor, g_dram.offset,
            [[GC, BC], [DSUB * BC * GC, DBLK], [BC * GC, DSUB]],
        )
        nc.sync.dma_start(g00[:], g00_src)
        nc.vector.scalar_tensor_tensor(
            out=carry0[:].rearrange("p (s m) -> p s m", m=M),
            in0=mv[:].rearrange("p (s m) -> p s m", m=M),
            scalar=1.0,
            in1=g00[:].unsqueeze(2).broadcast_to([P, DSUB, M]),
            op0=OP.mult, op1=OP.subtract,
        )

        for ch in range(NCH):
            cur = ch % 2
            t0 = ch * T
            # ---- chunk loads
            # w: [P, (j, m)] (replicated over dblk partitions via step-0)
            w_src = bass.AP(
                w_dram.tensor, w_dram.offset + t0 * M,
                [[L * M, BC], [0, DBLK], [1, T * M]],
            )
            nc.sync.dma_start(wt[cur][:].rearrange("p (f) -> p f"), w_src)
            # e: [P, (d_loc, j)]
            e_src = bass.AP(
                e_dram.tensor, e_dram.offset + t0,
                [[L, BC], [DSUB * NTP, DBLK], [NTP, DSUB], [1, T]],
            )
            nc.sync.dma_start(et[cur][:].rearrange("p (s j) -> p s j", j=T), e_src)
            # gamma: [P, (d_loc, j)] cols t0 .. t0+T (CL of them)
            g_src = bass.AP(
                g_dram.tensor, g_dram.offset + t0,
                [[GC, BC], [DSUB * BC * GC, DBLK], [BC * GC, DSUB], [1, CL]],
            )
            nc.sync.dma_start(gt[cur][:].rearrange("p (s j) -> p s j", j=CL), g_src)

            A_4 = A[:].rearrange("p (s m l) -> p s m l", m=M, l=CL)
            DL_4 = DL[cur][:].rearrange("p (s m l) -> p s m l", m=M, l=CL)
            S_4 = S[:].rearrange("p (s m l) -> p s m l", m=M, l=CL)
            w_mj = wt[cur][:].rearrange("p (j m) -> p m j", m=M)
            e_sj = et[cur][:].rearrange("p (s j) -> p s j", j=T)
            g_v = gt[cur][:].rearrange("p (s j) -> p s j", j=CL)

            # carry slot (slot 0 of every chain) of data1: for chunk 0 it is
            # written before the loop; for later chunks at the end of the
            # previous chunk (from S's last slots).
            if ch == 0:
                dl_slot0 = DL[0][:].rearrange("p (c l) -> p c l", l=CL)[:, :, 0:1]
                nc.scalar.copy(dl_slot0, carry0[:].unsqueeze(2))

            # A = 1 - w*e   (stt computes w*e per d_loc; ACT flips to 1 - x
            # in place over the whole buffer).  walrus limits stt/ACT APs to
            # 2 free dims, hence the per-d_loc split.
            for s in range(DSUB):
                nc.vector.scalar_tensor_tensor(
                    out=A_4[:, s, :, 1:], in0=w_mj, scalar=1.0,
                    in1=e_sj[:, s : s + 1, :].broadcast_to([P, M, T]),
                    op0=OP.mult, op1=OP.mult,
                )
            A_slots = A[:].rearrange("p (c l) -> p c l", l=CL)[:, :, 1:]
            nc.scalar.activation(A_slots, A_slots, AF.Copy, scale=-1.0, bias=1.0)

            # delta = gamma_{t-1} - gamma_t, replicated over m into data1
            ds_v = ds[cur][:].rearrange("p (s j) -> p s j", j=T)
            nc.vector.tensor_tensor(
                out=ds_v, in0=g_v[:, :, 0:T], in1=g_v[:, :, 1:CL], op=OP.subtract
            )
            for s in range(DSUB):
                nc.scalar.copy(
                    DL_4[:, s, :, 1:],
                    ds_v[:, s : s + 1, :].broadcast_to([P, M, T]),
                )

            # the scan
            nc.vector.tensor_tensor_scan(
                out=S[:], data0=A[:], data1=DL[cur][:],
                initial=0.0, op0=OP.mult, op1=OP.add,
            )

            # carry for the next chunk: last slot of each chain -> slot 0 of
            # the other DL buffer
            if ch < NCH - 1:
                dl_next0 = DL[1 - cur][:].rearrange("p (c l) -> p c l", l=CL)[:, :, 0:1]
                s_last = S[:].rearrange("p (c l) -> p c l", l=CL)[:, :, T : T + 1]
                nc.scalar.copy(dl_next0, s_last)

            # read_t = sum_m w_t * s~_{t-1} + gamma_{t-1}
            for s in range(DSUB):
                ws_s = DL[cur][:, s * M * T : (s + 1) * M * T].rearrange(
                    "p (m j) -> p m j", j=T
                )
                nc.vector.tensor_tensor(
                    out=ws_s, in0=S_4[:, s, :, 0:T], in1=w_mj, op=OP.mult
                )
            ws_r = DL[cur][:, : DSUB * M * T].rearrange("p (s m j) -> p s j m", m=M, j=T)
            rr_v = rr[cur][:].rearrange("p (s j) -> p s j", j=T)
            nc.vector.tensor_reduce(out=rr_v, in_=ws_r, axis=mybir.AxisListType.X, op=OP.add)
            nc.vector.tensor_tensor(out=rr_v, in0=rr_v, in1=g_v[:, :, 0:T], op=OP.add)

            rd_dst = bass.AP(
                rd_dram.tensor, rd_dram.offset + t0,
                [[L, BC], [DSUB * NTP, DBLK], [NTP, DSUB], [1, T]],
            )
            nc.sync.dma_start(rd_dst, rr_v)

    # ---------------- phase 4 ----------------
    with (
        tc.tile_pool(name="p4w", bufs=1) as wp4,
        tc.tile_pool(name="p4", bufs=3) as pool4,
        tc.tile_pool(name="p4ps", bufs=2, space="PSUM") as pp4,
    ):
        fwr_sb = wp4.tile([D, D], FP32, tag="fwr")
        nc.sync.dma_start(fwr_sb[:], io["fWr"])
        fwk_sb = wp4.tile([D, D], FP32, tag="fwk")
        nc.sync.dma_start(fwk_sb[:], io["fWk"])
        fb_sb = wp4.tile([1, D], FP32, tag="fb")
        nc.sync.dma_start(fb_sb[:], io["fb"])
        abw_sb = wp4.tile([D, 1], FP32, tag="abw")
        nc.sync.dma_start(abw_sb[:], io["abW"])
        dw_sb = wp4.tile([D, 1], FP32, tag="dw")
        nc.sync.dma_start(dw_sb[:], io["dW"])
        abb_sb = wp4.tile([P, 1], FP32, tag="abb")
        nc.sync.dma_start(abb_sb[:], io["abb"])
        dbb_sb = wp4.tile([P, 1], FP32, tag="dbb")
        nc.sync.dma_start(dbb_sb[:], io["dbb"])

        for i in range(NTILE):
            g0 = i * P
            kt4 = pool4.tile([D, P], FP32, tag="kt4")
            nc.sync.dma_start(kt4[:], kT_dram[:, g0 : g0 + P])
            rt4 = pool4.tile([D, P], FP32, tag="rt4")
            nc.sync.dma_start(rt4[:], rd_dram[:, g0 : g0 + P])
            f_ps = pp4.tile([D, P], FP32, tag="fps")
            nc.tensor.matmul(f_ps[:], lhsT=fwr_sb[:], rhs=rt4[:], start=True, stop=False)
            nc.tensor.matmul(f_ps[:], lhsT=fwk_sb[:], rhs=kt4[:], start=False, stop=False)
            nc.tensor.matmul(f_ps[:], lhsT=fb_sb[:], rhs=ones[:], start=False, stop=True)
            f_sb = pool4.tile([D, P], FP32, tag="fsb")
            nc.scalar.activation(f_sb[:], f_ps[:], AF.Tanh)
            ab_ps = pp4.tile([P, 1], FP32, tag="abps")
            nc.tensor.matmul(ab_ps[:], lhsT=f_sb[:], rhs=abw_sb[:], start=True, stop=True)
            qd_ps = pp4.tile([P, 1], FP32, tag="qdps")
            nc.tensor.matmul(qd_ps[:], lhsT=kt4[:], rhs=dw_sb[:], start=True, stop=True)
            abil = pool4.tile([P, 1], FP32, tag="abil")
            nc.scalar.activation(abil[:], ab_ps[:], AF.Tanh, bias=abb_sb[:])
            qd = pool4.tile([P, 1], FP32, tag="qd")
            nc.scalar.activation(qd[:], qd_ps[:], AF.Tanh, bias=dbb_sb[:])
            z = pool4.tile([P, 1], FP32, tag="z")
            nc.vector.scalar_tensor_tensor(
                out=z[:], in0=abil[:], scalar=3.0, in1=qd[:],
                op0=OP.mult, op1=OP.subtract,
            )
            pth = pool4.tile([P, 1], FP32, tag="pth")
            nc.scalar.activation(pth[:], z[:], AF.Tanh, scale=0.5)
            pt = pool4.tile([P, 1], FP32, tag="pt")
            nc.vector.tensor_scalar(
                out=pt[:], in0=pth[:], scalar1=0.5, scalar2=0.5,
                op0=OP.mult, op1=OP.add,
            )
            nc.sync.dma_start(io["p_out"][g0 : g0 + P, :], pt[:])


def _split_multi_waits(nc):
    """This walrus build allows only ONE sync-wait per instruction; move
    extras onto standalone InstEventSemaphore ops just before the
    instruction on the same engine (raw-bass style standalone waits)."""
    n = 0
    for fn in nc.m.functions:
        for blk in fn.blocks:
            new_list = []
            for inst in blk.instructions:
                si = inst.sync_info
                if si is not None and si.on_wait and len(si.on_wait) > 1:
                    for w in si.on_wait[:-1]:
                        n += 1
                        ev = mybir.InstEventSemaphore(
                            name=f"xwait_{n}_{inst.name}", ins=[], outs=[],
                            sync_info=mybir.SyncInfo(on_wait=[w], on_update=[]),
                        )
                        ev.engine = inst.engine
                        nc.register_instruction(ev, overwrite=True)
                        new_list.append(ev)
                    inst.sync_info = mybir.SyncInfo(
                        on_wait=[si.on_wait[-1]], on_update=si.on_update
                    )
                new_list.append(inst)
            blk.instructions[:] = new_list
    return n


def declare_io(nc, cfg):
    io = {}

    def inp(name, shape, dt=FP32):
        io[name] = nc.dram_tensor(name, shape, dt, kind="ExternalInput").ap()

    inp("kvidx", [cfg.ntp, 2], I32)
    inp("k_emb", [cfg.numc, cfg.d])
    inp("v_emb", [2 * cfg.numc, cfg.d])
    inp("MkT", [cfg.d, cfg.m])
    inp("eW", [cfg.d, cfg.d])
    inp("aW", [cfg.d, cfg.d])
    inp("eb", [1, cfg.d])
    inp("ab", [1, cfg.d])
    inp("fWr", [cfg.d, cfg.d])
    inp("fWk", [cfg.d, cfg.d])
    inp("fb", [1, cfg.d])
    inp("abW", [cfg.d, 1])
    inp("dW", [cfg.d, 1])
    inp("abb", [P, 1])
    inp("dbb", [P, 1])
    inp("Mv0", [cfg.m, cfg.d])
    io["p_out"] = nc.dram_tensor("p_out", [cfg.ntp, 1], FP32, kind="ExternalOutput").ap()
    return io


def build_nc(cfg=CFG):
    nc = bass.Bass("TRN2", num_devices=cfg.ncores)
    with tile.TileContext(nc) as tc:
        io = declare_io(nc, cfg)
        build_deepirt(tc, io, cfg)
    _split_multi_waits(nc)
    return nc


def host_prep(cfg, q, r, k_emb, v_emb, Mk, Mv0, e_W, e_b, a_W, a_b, f_W, f_b,
              ab_W, ab_b, d_W, d_b):
    """Returns per-core input maps."""
    q = np.asarray(q)
    r = np.asarray(r)
    shared = {
        "k_emb": np.ascontiguousarray(k_emb, np.float32),
        "v_emb": np.ascontiguousarray(v_emb, np.float32),
        "MkT": np.ascontiguousarray(np.asarray(Mk, np.float32).T),
        "eW": np.ascontiguousarray(e_W, np.float32),
        "aW": np.ascontiguousarray(a_W, np.float32),
        "eb": np.asarray(e_b, np.float32).reshape(1, cfg.d),
        "ab": np.asarray(a_b, np.float32).reshape(1, cfg.d),
        "fWr": np.ascontiguousarray(np.asarray(f_W, np.float32)[: cfg.d]),
        "fWk": np.ascontiguousarray(np.asarray(f_W, np.float32)[cfg.d :]),
        "fb": np.asarray(f_b, np.float32).reshape(1, cfg.d),
        "abW": np.asarray(ab_W, np.float32).reshape(cfg.d, 1),
        "dW": np.asarray(d_W, np.float32).reshape(cfg.d, 1),
        "abb": np.full((P, 1), np.float32(np.asarray(ab_b).reshape(-1)[0])),
        "dbb": np.full((P, 1), np.float32(np.asarray(d_b).reshape(-1)[0])),
        "Mv0": np.ascontiguousarray(Mv0, np.float32),
    }
    maps = []
    for c in range(cfg.ncores):
        bsl = slice(c * cfg.bc, (c + 1) * cfg.bc)
        kidx = q[bsl].reshape(-1).astype(np.int32)
        vidx = (q[bsl].astype(np.int64) + cfg.numc * r[bsl].astype(np.int64)).reshape(-1).astype(np.int32)
        kv = np.zeros((cfg.ntp, 2), np.int32)
        kv[: cfg.nt, 0] = kidx
        kv[: cfg.nt, 1] = vidx
        maps.append({"kvidx": kv, **shared})
    return maps


_NC_CACHE = {}


def kernel(**inputs):
    cfg = CFG
    if "nc" not in _NC_CACHE:
        _NC_CACHE["nc"] = build_nc(cfg)
    nc = _NC_CACHE["nc"]
    from concourse.bass_utils import run_bass_kernel_spmd

    maps = host_prep(cfg, **inputs)
    res = run_bass_kernel_spmd(nc, maps, core_ids=list(range(cfg.ncores)))
    outs = []
    for c in range(cfg.ncores):
        p = res.results[c]["p_out"].reshape(-1)[: cfg.nt].reshape(cfg.bc, cfg.l)
        outs.append(p)
    return np.concatenate(outs, axis=0).astype(np.float32)


# revision 17
# speedup vs baseline: 60.1515x; 1.2715x over previous
"""DeepIRT (nn_DeepIRT) Trainium2 Bass kernel.

Full inputs in, full output out. Internally: data-parallel over batch across
8 NeuronCores (16 batch rows per core), everything else on-device per core:

  phase 1: embedding gathers (indirect DMA), PE matmuls for logits/e/a,
           softmax, and gamma = a/e computed division-free as a*(1+exp(-z))
           (e = sigmoid(z) => 1/e = 1 + exp(-z)).
  phase 2: the sequential memory update
               Mv_t = Mv_{t-1} * (1 - w x e) + w x a
           is an elementwise *linear* recurrence per (b,m,d) cell, computed
           with the native DVE prefix-scan (tensor_tensor_scan).  The state
           shift  s~ = Mv - gamma  (gamma = a/e, per (b,d,t) only) makes the
           additive term independent of m:
               s~_t = s~_{t-1} * (1 - w_t e_t) + (gamma_{t-1} - gamma_t)
           and since softmax weights sum to one:
               read_t = sum_m w_t[m] s~_{t-1}[m,d] + gamma_{t-1}[d].
           Chains (one per (d,m) pair, 400 per partition) are packed along
           the free dim with one reset slot per chain per time-chunk to
           carry the state across chunks.
  phase 4: f/ability/que_diff matmuls on PE + final sigmoid.
"""

import math
import os
import sys

import numpy as np

for _p in ("/opt/trn_rl_repo",):
    if _p not in sys.path and os.path.isdir(_p):
        sys.path.insert(0, _p)

import concourse.bass as bass
import concourse.tile as tile
from concourse import mybir
from concourse._compat import with_exitstack
from concourse.bass import IndirectOffsetOnAxis
from concourse.masks import make_identity

AF = mybir.ActivationFunctionType
OP = mybir.AluOpType
FP32 = mybir.dt.float32
I32 = mybir.dt.int32

P = 128  # SBUF partitions


class Cfg:
    def __init__(self, numc=10000, d=64, m=50, l=500, t=25, ncores=8, b=128):
        self.numc, self.d, self.m, self.l, self.t = numc, d, m, l, t
        self.ncores, self.b = ncores, b
        self.bc = b // ncores                      # batch rows per core (16)
        assert P % self.bc == 0
        self.dblk = P // self.bc                   # d-blocks per batch row (8)
        assert d % self.dblk == 0
        self.dsub = d // self.dblk                 # d's per partition (8)
        self.nt = self.bc * l                      # tokens per core
        self.ntile = math.ceil(self.nt / P)
        self.ntp = self.ntile * P
        self.cl = t + 1                            # chain slots per chunk
        assert l % t == 0
        self.nch = l // t
        self.gc = l + 4                            # padded per-b gamma cols
        self.fbig = self.dsub * m * self.cl        # scan buffer elems/partition


CFG = Cfg()


def _split_tokens(g0, n, l):
    """Split token range [g0, g0+n) at batch-row boundaries.

    Yields (b, tlo, thi, j0): tokens g0+j0 .. map to (b, tlo..thi)."""
    j = 0
    while j < n:
        g = g0 + j
        b, t = divmod(g, l)
        cnt = min(n - j, l - t)
        yield b, t, t + cnt, j
        j += cnt


@with_exitstack
def build_deepirt(ctx, tc, io, cfg):
    nc = tc.nc
    D, M, L, T, BC = cfg.d, cfg.m, cfg.l, cfg.t, cfg.bc
    DBLK, DSUB, CL, NCH, GC = cfg.dblk, cfg.dsub, cfg.cl, cfg.nch, cfg.gc
    NT, NTILE, NTP = cfg.nt, cfg.ntile, cfg.ntp
    NTB = math.ceil(L / P)

    # internal DRAM scratch
    w_dram = nc.dram_tensor("w_scr", [NTP, M], FP32).ap()
    e_dram = nc.dram_tensor("e_scr", [D, NTP], FP32).ap()
    g_dram = nc.dram_tensor("g_scr", [D, BC * GC], FP32).ap()
    kT_dram = nc.dram_tensor("kT_scr", [D, NTP], FP32).ap()
    rd_dram = nc.dram_tensor("rd_scr", [D, NTP], FP32).ap()
    g_dram_v = g_dram.rearrange("d (b c) -> d b c", b=BC)

    cpool = ctx.enter_context(tc.tile_pool(name="const", bufs=1))
    ident = cpool.tile([P, P], FP32, tag="ident")
    make_identity(nc, ident[:])
    ones = cpool.tile([1, P], FP32, tag="ones")
    nc.vector.memset(ones[:], 1.0)
    zeros = cpool.tile([D, P], FP32, tag="zeros")
    nc.vector.memset(zeros[:], 0.0)

    wp = ctx.enter_context(tc.tile_pool(name="p1w", bufs=1))
    pool = ctx.enter_context(tc.tile_pool(name="p1", bufs=4))
    pp = ctx.enter_context(tc.tile_pool(name="p1ps", bufs=1, space="PSUM"))
    sp = ctx.enter_context(tc.tile_pool(name="scan", bufs=1))
    wp4 = ctx.enter_context(tc.tile_pool(name="p4w", bufs=1))
    pool4 = ctx.enter_context(tc.tile_pool(name="p4", bufs=3))
    pp4 = ctx.enter_context(tc.tile_pool(name="p4ps", bufs=1, space="PSUM"))

    # ---- static loads
    mkt_sb = wp.tile([D, M], FP32, tag="mkt")
    nc.sync.dma_start(mkt_sb[:], io["MkT"])
    ew_sb = wp.tile([D, D], FP32, tag="ew")
    nc.sync.dma_start(ew_sb[:], io["eW"])
    aw_sb = wp.tile([D, D], FP32, tag="aw")
    nc.sync.dma_start(aw_sb[:], io["aW"])
    eb_sb = wp.tile([1, D], FP32, tag="eb")
    nc.sync.dma_start(eb_sb[:], io["eb"])
    ab_sb = wp.tile([1, D], FP32, tag="ab")
    nc.sync.dma_start(ab_sb[:], io["ab"])
    fwr_sb = wp4.tile([D, D], FP32, tag="fwr")
    nc.sync.dma_start(fwr_sb[:], io["fWr"])
    fwk_sb = wp4.tile([D, D], FP32, tag="fwk")
    nc.sync.dma_start(fwk_sb[:], io["fWk"])
    fb_sb = wp4.tile([1, D], FP32, tag="fb")
    nc.sync.dma_start(fb_sb[:], io["fb"])
    abw_sb = wp4.tile([D, 1], FP32, tag="abw")
    nc.sync.dma_start(abw_sb[:], io["abW"])
    dw_sb = wp4.tile([D, 1], FP32, tag="dw")
    nc.sync.dma_start(dw_sb[:], io["dW"])
    abb_sb = wp4.tile([P, 1], FP32, tag="abb")
    nc.sync.dma_start(abb_sb[:], io["abb"])
    dbb_sb = wp4.tile([P, 1], FP32, tag="dbb")
    nc.sync.dma_start(dbb_sb[:], io["dbb"])

    # all gather indices in one DMA: idx_all[p, (u, c)] = kvidx[u*128+p, c]
    idx_all = wp.tile([P, NTB * BC * 2], I32, tag="idxall")
    idx_src = bass.AP(io["kvidx"].tensor, io["kvidx"].offset, [[2, P], [2 * P, NTB * BC], [1, 2]])
    nc.sync.dma_start(idx_all[:].rearrange("p (u c) -> p u c", c=2), idx_src)

    # zero pads: gamma tail cols per b, kT/reads pad cols
    nc.sync.dma_start(g_dram_v[:, :, L:GC], zeros[:, : BC * (GC - L)].rearrange("d (b c) -> d b c", b=BC))
    if NTP > NT:
        nc.sync.dma_start(kT_dram[:, NT:NTP], zeros[:, : NTP - NT])
        nc.sync.dma_start(rd_dram[:, NT:NTP], zeros[:, : NTP - NT])

    # ---- scan-phase persistent tiles
    A = sp.tile([P, cfg.fbig], FP32, tag="A", name="A")
    DL = sp.tile([P, cfg.fbig], FP32, tag="DL", name="DL")
    S = sp.tile([P, cfg.fbig], FP32, tag="S", name="S")
    wt = [sp.tile([P, T * M], FP32, tag=f"wt{i}", name=f"wt{i}") for i in range(2)]
    et = [sp.tile([P, DSUB * T], FP32, tag=f"et{i}", name=f"et{i}") for i in range(2)]
    gt = [sp.tile([P, DSUB * CL], FP32, tag=f"gt{i}", name=f"gt{i}") for i in range(2)]
    ds = [sp.tile([P, DSUB * T], FP32, tag=f"ds{i}", name=f"ds{i}") for i in range(2)]
    rr = [sp.tile([P, DSUB * T], FP32, tag=f"rr{i}", name=f"rr{i}") for i in range(2)]
    mv = sp.tile([P, DSUB * M], FP32, tag="mv")
    g00 = sp.tile([P, DSUB], FP32, tag="g00")
    carry0 = sp.tile([P, DSUB * M], FP32, tag="c0")

    nc.vector.memset(A[:].rearrange("p (c l) -> p c l", l=CL)[:, :, 0:1], 0.0)
    mv0_v = io["Mv0"].rearrange("m (g s) -> g s m", s=DSUB)  # [DBLK, DSUB, M]
    for b in range(BC):
        nc.sync.dma_start(
            mv[b * DBLK : (b + 1) * DBLK, :].rearrange("p (s m) -> p s m", m=M),
            mv0_v,
        )

    # ---------------- phase-1 tile ----------------
    def emit_p1(b, tb):
        u = tb * BC + b
        t0 = tb * P
        n = min(P, L - t0)
        c0 = b * L + t0
        k_tok = pool.tile([P, D], FP32, tag="ktok", name="ktok")
        nc.gpsimd.indirect_dma_start(
            out=k_tok[:], out_offset=None, in_=io["k_emb"],
            in_offset=IndirectOffsetOnAxis(ap=idx_all[:, 2 * u : 2 * u + 1], axis=0),
        )
        v_tok = pool.tile([P, D], FP32, tag="vtok", name="vtok")
        nc.gpsimd.indirect_dma_start(
            out=v_tok[:], out_offset=None, in_=io["v_emb"],
            in_offset=IndirectOffsetOnAxis(ap=idx_all[:, 2 * u + 1 : 2 * u + 2], axis=0),
        )
        kt_ps = pp.tile([D, P], FP32, tag="ktps")
        nc.tensor.matmul(kt_ps[:], lhsT=k_tok[:], rhs=ident[:], start=True, stop=True)
        kt = pool.tile([D, P], FP32, tag="kt", name="kt")
        nc.scalar.copy(kt[:], kt_ps[:])
        vt_ps = pp.tile([D, P], FP32, tag="vtps")
        nc.tensor.matmul(vt_ps[:], lhsT=v_tok[:], rhs=ident[:], start=True, stop=True)
        vt = pool.tile([D, P], FP32, tag="vt", name="vt")
        nc.scalar.copy(vt[:], vt_ps[:])
        nc.sync.dma_start(kT_dram[:, c0 : c0 + n], kt[:, :n])

        lg_ps = pp.tile([P, M], FP32, tag="lg")
        nc.tensor.matmul(lg_ps[:], lhsT=kt[:], rhs=mkt_sb[:], start=True, stop=True)
        wexp = pool.tile([P, M], FP32, tag="wexp", name="wexp")
        wsum = pool.tile([P, 1], FP32, tag="wsum", name="wsum")
        nc.scalar.activation(wexp[:], lg_ps[:], AF.Exp, accum_out=wsum[:])
        wrec = pool.tile([P, 1], FP32, tag="wrec", name="wrec")
        nc.vector.reciprocal(wrec[:], wsum[:])
        wn = pool.tile([P, M], FP32, tag="wn", name="wn")
        nc.scalar.activation(wn[:], wexp[:], AF.Copy, scale=wrec[:])
        nc.sync.dma_start(w_dram[c0 : c0 + n, :], wn[:n, :])

        e_ps = pp.tile([D, P], FP32, tag="eps")
        nc.tensor.matmul(e_ps[:], lhsT=ew_sb[:], rhs=vt[:], start=True, stop=False)
        nc.tensor.matmul(e_ps[:], lhsT=eb_sb[:], rhs=ones[:], start=False, stop=True)
        # sigmoid via tanh so every ACT func stays in the exp_and_others table
        e_th = pool.tile([D, P], FP32, tag="eth", name="eth")
        nc.scalar.activation(e_th[:], e_ps[:], AF.Tanh, scale=0.5)
        e_sb = pool.tile([D, P], FP32, tag="esb", name="esb")
        nc.scalar.activation(e_sb[:], e_th[:], AF.Copy, scale=0.5, bias=0.5)
        xz = pool.tile([D, P], FP32, tag="xz", name="xz")
        nc.scalar.activation(xz[:], e_ps[:], AF.Exp, scale=-1.0)
        a_ps = pp.tile([D, P], FP32, tag="aps")
        nc.tensor.matmul(a_ps[:], lhsT=aw_sb[:], rhs=vt[:], start=True, stop=False)
        nc.tensor.matmul(a_ps[:], lhsT=ab_sb[:], rhs=ones[:], start=False, stop=True)
        a_sb = pool.tile([D, P], FP32, tag="asb", name="asb")
        nc.scalar.activation(a_sb[:], a_ps[:], AF.Tanh)
        g_sb = pool.tile([D, P], FP32, tag="gsb", name="gsb")
        nc.vector.scalar_tensor_tensor(
            out=g_sb[:], in0=xz[:], scalar=1.0, in1=a_sb[:],
            op0=OP.add, op1=OP.mult,
        )
        nc.sync.dma_start(e_dram[:, c0 : c0 + n], e_sb[:, :n])
        nc.sync.dma_start(g_dram_v[:, b, t0 : t0 + n], g_sb[:, :n])

    # ---------------- phase-4 tile ----------------
    def emit_p4(c0, n):
        kt4 = pool4.tile([D, P], FP32, tag="kt4", name="kt4")
        nc.sync.dma_start(kt4[:, :n], kT_dram[:, c0 : c0 + n])
        rt4 = pool4.tile([D, P], FP32, tag="rt4", name="rt4")
        nc.sync.dma_start(rt4[:, :n], rd_dram[:, c0 : c0 + n])
        f_ps = pp4.tile([D, P], FP32, tag="fps")
        nc.tensor.matmul(f_ps[:, :n], lhsT=fwr_sb[:], rhs=rt4[:, :n], start=True, stop=False)
        nc.tensor.matmul(f_ps[:, :n], lhsT=fwk_sb[:], rhs=kt4[:, :n], start=False, stop=False)
        nc.tensor.matmul(f_ps[:, :n], lhsT=fb_sb[:], rhs=ones[:, :n], start=False, stop=True)
        f_sb = pool4.tile([D, P], FP32, tag="fsb", name="fsb")
        nc.scalar.activation(f_sb[:, :n], f_ps[:, :n], AF.Tanh)
        ab_ps = pp4.tile([P, 1], FP32, tag="abps")
        nc.tensor.matmul(ab_ps[:n, :], lhsT=f_sb[:, :n], rhs=abw_sb[:], start=True, stop=True)
        qd_ps = pp4.tile([P, 1], FP32, tag="qdps")
        nc.tensor.matmul(qd_ps[:n, :], lhsT=kt4[:, :n], rhs=dw_sb[:], start=True, stop=True)
        abil = pool4.tile([P, 1], FP32, tag="abil", name="abil")
        nc.scalar.activation(abil[:n, :], ab_ps[:n, :], AF.Tanh, bias=abb_sb[:n, :])
        qd = pool4.tile([P, 1], FP32, tag="qd", name="qd")
        nc.scalar.activation(qd[:n, :], qd_ps[:n, :], AF.Tanh, bias=dbb_sb[:n, :])
        z = pool4.tile([P, 1], FP32, tag="z", name="z")
        nc.vector.scalar_tensor_tensor(
            out=z[:n, :], in0=abil[:n, :], scalar=3.0, in1=qd[:n, :],
            op0=OP.mult, op1=OP.subtract,
        )
        pth = pool4.tile([P, 1], FP32, tag="pth", name="pth")
        nc.scalar.activation(pth[:n, :], z[:n, :], AF.Tanh, scale=0.5)
        pt = pool4.tile([P, 1], FP32, tag="pt", name="pt")
        nc.scalar.activation(pt[:n, :], pth[:n, :], AF.Copy, scale=0.5, bias=0.5)
        nc.sync.dma_start(io["p_out"][c0 : c0 + n, :], pt[:n, :])

    # ---------------- scan chunk ----------------
    def emit_chunk(ch):
        cur = ch % 2
        t0 = ch * T
        w_src = bass.AP(
            w_dram.tensor, w_dram.offset + t0 * M,
            [[L * M, BC], [0, DBLK], [1, T * M]],
        )
        nc.sync.dma_start(wt[cur][:].rearrange("p (f) -> p f"), w_src)
        e_src = bass.AP(
            e_dram.tensor, e_dram.offset + t0,
            [[L, BC], [DSUB * NTP, DBLK], [NTP, DSUB], [1, T]],
        )
        nc.sync.dma_start(et[cur][:].rearrange("p (s j) -> p s j", j=T), e_src)
        g_src = bass.AP(
            g_dram.tensor, g_dram.offset + t0,
            [[GC, BC], [DSUB * BC * GC, DBLK], [BC * GC, DSUB], [1, CL]],
        )
        nc.sync.dma_start(gt[cur][:].rearrange("p (s j) -> p s j", j=CL), g_src)

        A_4 = A[:].rearrange("p (s m l) -> p s m l", m=M, l=CL)
        DL_4 = DL[:].rearrange("p (s m l) -> p s m l", m=M, l=CL)
        S_4 = S[:].rearrange("p (s m l) -> p s m l", m=M, l=CL)
        w_mj = wt[cur][:].rearrange("p (j m) -> p m j", m=M)
        e_sj = et[cur][:].rearrange("p (s j) -> p s j", j=T)
        g_v = gt[cur][:].rearrange("p (s j) -> p s j", j=CL)

        if ch == 0:
            dl_slot0 = DL[:].rearrange("p (c l) -> p c l", l=CL)[:, :, 0:1]
            nc.scalar.copy(dl_slot0, carry0[:].unsqueeze(2))

        # delta = gamma_{t-1} - gamma_t, replicated over m into data1
        ds_v = ds[cur][:].rearrange("p (s j) -> p s j", j=T)
        nc.vector.tensor_tensor(
            out=ds_v, in0=g_v[:, :, 0:T], in1=g_v[:, :, 1:CL], op=OP.subtract
        )
        for s in range(DSUB):
            nc.scalar.copy(
                DL_4[:, s, :, 1:],
                ds_v[:, s : s + 1, :].broadcast_to([P, M, T]),
            )

        # A = 1 - w*e: per-d_loc stt (DVE) + trailing in-place ACT flip
        for s in range(DSUB):
            nc.vector.scalar_tensor_tensor(
                out=A_4[:, s, :, 1:], in0=w_mj, scalar=1.0,
                in1=e_sj[:, s : s + 1, :].broadcast_to([P, M, T]),
                op0=OP.mult, op1=OP.mult,
            )
            nc.scalar.activation(
                A_4[:, s, :, 1:], A_4[:, s, :, 1:], AF.Copy, scale=-1.0, bias=1.0
            )

        nc.vector.tensor_tensor_scan(
            out=S[:], data0=A[:], data1=DL[:],
            initial=0.0, op0=OP.mult, op1=OP.add,
        )

        # carry for the next chunk -> slot 0 of every chain in DL (safe
        # right after the scan: nothing below touches DL slot 0)
        if ch < NCH - 1:
            dl_next0 = DL[:].rearrange("p (c l) -> p c l", l=CL)[:, :, 0:1]
            s_last = S[:].rearrange("p (c l) -> p c l", l=CL)[:, :, T : T + 1]
            nc.scalar.copy(dl_next0, s_last)

        # read_t = sum_m w_t * s~_{t-1} + gamma_{t-1}.  WS goes into the A
        # buffer's slot>=1 positions (A is dead after the scan and the
        # slot-0 zeros stay intact), so DL is free for the next chunk's
        # delta replication immediately after the scan.
        for s in range(DSUB):
            nc.vector.tensor_tensor(
                out=A_4[:, s, :, 1:], in0=S_4[:, s, :, 0:T], in1=w_mj, op=OP.mult
            )
        ws_r = A[:].rearrange("p (s m l) -> p s l m", m=M, l=CL)[:, :, 1:, :]
        rr_v = rr[cur][:].rearrange("p (s j) -> p s j", j=T)
        nc.vector.tensor_reduce(out=rr_v, in_=ws_r, axis=mybir.AxisListType.X, op=OP.add)
        nc.vector.tensor_tensor(out=rr_v, in0=rr_v, in1=g_v[:, :, 0:T], op=OP.add)

        rd_dst = bass.AP(
            rd_dram.tensor, rd_dram.offset + t0,
            [[L, BC], [DSUB * NTP, DBLK], [NTP, DSUB], [1, T]],
        )
        nc.sync.dma_start(rd_dst, rr_v)

    # ---------------- interleaved emission ----------------
    chunks_of_tb = {}
    for ch in range(NCH):
        need = min(T * ch + T, L - 1)
        tb = need // P
        chunks_of_tb.setdefault(tb, []).append(ch)
    p4_by_chunk = {}
    for b in range(BC):
        tb0 = 0
        while tb0 < L:
            n = min(P, L - tb0)
            ready = (tb0 + n - 1) // T
            p4_by_chunk.setdefault(ready, []).append((b * L + tb0, n))
            tb0 += n

    for tb in range(NTB):
        for b in range(BC):
            emit_p1(b, tb)
        if tb == 0:
            g00_src = bass.AP(
                g_dram.tensor, g_dram.offset,
                [[GC, BC], [DSUB * BC * GC, DBLK], [BC * GC, DSUB]],
            )
            nc.sync.dma_start(g00[:], g00_src)
            nc.vector.scalar_tensor_tensor(
                out=carry0[:].rearrange("p (s m) -> p s m", m=M),
                in0=mv[:].rearrange("p (s m) -> p s m", m=M),
                scalar=1.0,
                in1=g00[:].unsqueeze(2).broadcast_to([P, DSUB, M]),
                op0=OP.mult, op1=OP.subtract,
            )
        for ch in chunks_of_tb.get(tb, []):
            emit_chunk(ch)
            for c0, n in p4_by_chunk.get(ch, []):
                emit_p4(c0, n)


def _split_multi_waits(nc):
    """This walrus build allows only ONE sync-wait per instruction; move
    extras onto standalone InstEventSemaphore ops just before the
    instruction on the same engine (raw-bass style standalone waits)."""
    n = 0
    for fn in nc.m.functions:
        for blk in fn.blocks:
            new_list = []
            for inst in blk.instructions:
                si = inst.sync_info
                if si is not None and si.on_wait and len(si.on_wait) > 1:
                    for w in si.on_wait[:-1]:
                        n += 1
                        ev = mybir.InstEventSemaphore(
                            name=f"xwait_{n}_{inst.name}", ins=[], outs=[],
                            sync_info=mybir.SyncInfo(on_wait=[w], on_update=[]),
                        )
                        ev.engine = inst.engine
                        nc.register_instruction(ev, overwrite=True)
                        new_list.append(ev)
                    inst.sync_info = mybir.SyncInfo(
                        on_wait=[si.on_wait[-1]], on_update=si.on_update
                    )
                new_list.append(inst)
            blk.instructions[:] = new_list
    return n


def declare_io(nc, cfg):
    io = {}

    def inp(name, shape, dt=FP32):
        io[name] = nc.dram_tensor(name, shape, dt, kind="ExternalInput").ap()

    ntb = math.ceil(cfg.l / P)
    inp("kvidx", [ntb * cfg.bc * P, 2], I32)
    inp("k_emb", [cfg.numc, cfg.d])
    inp("v_emb", [2 * cfg.numc, cfg.d])
    inp("MkT", [cfg.d, cfg.m])
    inp("eW", [cfg.d, cfg.d])
    inp("aW", [cfg.d, cfg.d])
    inp("eb", [1, cfg.d])
    inp("ab", [1, cfg.d])
    inp("fWr", [cfg.d, cfg.d])
    inp("fWk", [cfg.d, cfg.d])
    inp("fb", [1, cfg.d])
    inp("abW", [cfg.d, 1])
    inp("dW", [cfg.d, 1])
    inp("abb", [P, 1])
    inp("dbb", [P, 1])
    inp("Mv0", [cfg.m, cfg.d])
    io["p_out"] = nc.dram_tensor("p_out", [cfg.ntp, 1], FP32, kind="ExternalOutput").ap()
    return io


def build_nc(cfg=CFG):
    nc = bass.Bass("TRN2", num_devices=cfg.ncores)
    with tile.TileContext(nc) as tc:
        io = declare_io(nc, cfg)
        build_deepirt(tc, io, cfg)
    _split_multi_waits(nc)
    return nc


def host_prep(cfg, q, r, k_emb, v_emb, Mk, Mv0, e_W, e_b, a_W, a_b, f_W, f_b,
              ab_W, ab_b, d_W, d_b):
    """Returns per-core input maps."""
    q = np.asarray(q)
    r = np.asarray(r)
    shared = {
        "k_emb": np.ascontiguousarray(k_emb, np.float32),
        "v_emb": np.ascontiguousarray(v_emb, np.float32),
        "MkT": np.ascontiguousarray(np.asarray(Mk, np.float32).T),
        "eW": np.ascontiguousarray(e_W, np.float32),
        "aW": np.ascontiguousarray(a_W, np.float32),
        "eb": np.asarray(e_b, np.float32).reshape(1, cfg.d),
        "ab": np.asarray(a_b, np.float32).reshape(1, cfg.d),
        "fWr": np.ascontiguousarray(np.asarray(f_W, np.float32)[: cfg.d]),
        "fWk": np.ascontiguousarray(np.asarray(f_W, np.float32)[cfg.d :]),
        "fb": np.asarray(f_b, np.float32).reshape(1, cfg.d),
        "abW": np.asarray(ab_W, np.float32).reshape(cfg.d, 1),
        "dW": np.asarray(d_W, np.float32).reshape(cfg.d, 1),
        "abb": np.full((P, 1), np.float32(np.asarray(ab_b).reshape(-1)[0])),
        "dbb": np.full((P, 1), np.float32(np.asarray(d_b).reshape(-1)[0])),
        "Mv0": np.ascontiguousarray(Mv0, np.float32),
    }
    maps = []
    ntb = math.ceil(cfg.l / P)
    for c in range(cfg.ncores):
        bsl = slice(c * cfg.bc, (c + 1) * cfg.bc)
        kidx = q[bsl].astype(np.int64)
        vidx = kidx + cfg.numc * r[bsl].astype(np.int64)
        # tile u = tb*BC + b covers tokens (b, tb*128 .. tb*128+n)
        kv = np.zeros((ntb * cfg.bc * P, 2), np.int32)
        u = 0
        for tb in range(ntb):
            t0 = tb * P
            n = min(P, cfg.l - t0)
            for b in range(cfg.bc):
                kv[u * P : u * P + n, 0] = kidx[b, t0 : t0 + n]
                kv[u * P : u * P + n, 1] = vidx[b, t0 : t0 + n]
                u += 1
        maps.append({"kvidx": kv, **shared})
    return maps


_NC_CACHE = {}


def kernel(**inputs):
    cfg = CFG
    if "nc" not in _NC_CACHE:
        _NC_CACHE["nc"] = build_nc(cfg)
    nc = _NC_CACHE["nc"]
    from concourse.bass_utils import run_bass_kernel_spmd

    maps = host_prep(cfg, **inputs)
    res = run_bass_kernel_spmd(nc, maps, core_ids=list(range(cfg.ncores)))
    outs = []
    for c in range(cfg.ncores):
        p = res.results[c]["p_out"].reshape(-1)[: cfg.nt].reshape(cfg.bc, cfg.l)
        outs.append(p)
    return np.concatenate(outs, axis=0).astype(np.float32)
